# revision 11
# baseline (speedup 1.0000x reference)
"""nn_DecoderAutoregAdaIN on TRN2 (single core), v2.

Algorithm (validated in proto.py):
  - Cross-attn is diagonal => precomputed per-layer constant ca_add.
  - KV-cache incremental decode over 64 steps.
  - Rank-64 layer-0 QKV: qkv0 = W0 @ row + tab0[i], fused with emb = mm(row);
    row is the 64-dim motion vector fed back each step.
  - Deferred LN: m = (z - mu) feeds the next matmul; rstd applied to the
    matmul output (per-(b) scalar). rstd = exp(-0.5*ln(var+eps)) so the ACT
    engine never leaves the natural_log_exp table set (no table reloads).
  - Softmax: 1/S folded into the P-transpose identity (diag(1/S)).

Layouts (activations feature-major; feature f = c*128 + p, head h = 2c + (p>=64)):
  x / residuals  [128, (4c, 2b)] fp32
  qkvb           [128, (12ch, 2b)] bf16   ch 0-3 q, 4-7 k, 8-11 v
  KT cache       [128, (L, 4kc, 2b, 64t)] bf16
  V_row          [128=(2b,64t), (L, 512d)] bf16 via V_ps PSUM accumulation
  scores         [16, 64]  slot s(b,h) = 4*(h//2) + 2*b + (h%2)
  qblock         [128, (8e, 16s)] bf16, block e = 2c+b, flat = 18*(2c+b)+hpar
  row            [64, (2b)] bf16
"""
from contextlib import ExitStack
import numpy as np
import ml_dtypes

import concourse.bass as bass
from concourse import mybir
from concourse.alu_op_type import AluOpType as ALU

F32 = mybir.dt.float32
BF16 = mybir.dt.bfloat16
AX = mybir.AxisListType.X
ACTF = mybir.ActivationFunctionType

B, T, D, M, H, L, DFF, PERIOD = 2, 64, 512, 64, 8, 3, 2048, 30
HD = D // H
EPS = 1e-5
NCK = 4          # feature chunks of 128
NF = DFF // 128  # 16
SQH = 1.0 / np.sqrt(HD)

N_REPEAT = 1     # outer timing-loop repeats (bench builds >1)


def slot_of(b, h):
    return 4 * (h // 2) + 2 * b + (h % 2)


# ---------------------------------------------------------------- host prep
def _slopes(n):
    start = 2.0 ** (-(2.0 ** -(np.log2(n) - 3)))
    return np.array([start * start ** i for i in range(n)], dtype=np.float32)


def _pe_mask():
    pos = np.arange(PERIOD)[:, None].astype(np.float32)
    div = np.exp(np.arange(0, D, 2).astype(np.float32) * (-np.log(10000.0) / D))
    pe = np.zeros((PERIOD, D), np.float32)
    pe[:, 0::2] = np.sin(pos * div)
    pe[:, 1::2] = np.cos(pos * div)
    pe_full = np.tile(pe, (T // PERIOD + 1, 1))[:T]
    ii = np.arange(T)[:, None]
    jj = np.arange(T)[None, :]
    bias = -((ii - jj) // PERIOD).astype(np.float32)
    alibi = _slopes(H)[:, None, None] * np.where(jj <= ii, bias, 0.0)
    self_mask = np.where(jj <= ii, alibi, -1e9).astype(np.float32)  # [H,T,T]
    return pe_full, self_mask


def _wtiles(w_t, n_kc):
    """w_t [K, Mo] -> [128, n_kc, Mo]; lhsT tile (kc, mc) = arr[:, kc, mc*128:(mc+1)*128]."""
    K, Mo = w_t.shape
    assert K == n_kc * 128
    return np.ascontiguousarray(w_t.reshape(n_kc, 128, Mo).transpose(1, 0, 2))


def _bf(x):
    return np.ascontiguousarray(np.asarray(x).astype(ml_dtypes.bfloat16))


def prep_inputs(inp):
    inp = {k: np.asarray(v, np.float32) for k, v in inp.items()}
    # this kernel build assumes the reference's zero biases / identity LN affine
    for k in ("sa_b", "sa_o_b", "ca_o_b", "ff1_b", "ff2_b", "mm_b", "mmr_b",
              "adain_b", "ln_b"):
        assert np.all(inp[k] == 0.0), f"nonzero {k} unsupported by this build"
    assert np.all(inp["ln_g"] == 1.0), "non-identity ln_g unsupported"
    assert np.all(inp["ca_b"][:, 2 * D:] == 0.0), "nonzero ca v-bias unsupported"

    pe_full, self_mask = _pe_mask()
    out = {}

    # q-part scaled by 1/sqrt(HD)
    sa_w = inp["sa_w"].copy()
    sa_w[:, :D] *= SQH

    out["w_qkv"] = _bf(np.stack([_wtiles(sa_w[l].T, NCK) for l in range(L)], axis=1))
    out["w_out"] = _bf(np.stack([_wtiles(inp["sa_o_w"][l].T, NCK) for l in range(L)], axis=1))
    out["w_ff1"] = _bf(np.stack([_wtiles(inp["ff1_w"][l].T, NCK) for l in range(L)], axis=1))
    out["w_ff2"] = _bf(np.stack([_wtiles(inp["ff2_w"][l].T, NF) for l in range(L)], axis=1))
    out["w_cav"] = _bf(np.stack([_wtiles(inp["ca_w"][l][2 * D:].T, NCK) for l in range(L)], axis=1))
    out["w_cao"] = _bf(np.stack([_wtiles(inp["ca_o_w"][l].T, NCK) for l in range(L)], axis=1))
    out["w_adain"] = _bf(_wtiles(inp["adain_w"].T, NCK))      # [128, 4, 1024]
    out["w_mmr"] = _bf(_wtiles(inp["mmr_w"].T, NCK))          # [128, 4, 64]

    # fused layer-0 weight: row [64] -> (qkv0 [1536] | emb [512])
    w0 = np.concatenate([(sa_w[0] @ inp["mm_w"]).T, inp["mm_w"].T], axis=1)  # [64, 2048]
    out["w_row0"] = _bf(w0)                                   # [64, 2048]
    # tables: tab0[i] = sa_w0 @ (pe_i + mm_b) + sa_b0 (scaled q); pex[i] = pe_i + mm_b
    tab0 = (pe_full + inp["mm_b"][None, :]) @ sa_w[0].T       # [T, 1536]
    out["tab0_t"] = np.ascontiguousarray(tab0.T.reshape(12, 128, T).transpose(1, 0, 2))  # [128,12,T]
    pex = pe_full + inp["mm_b"][None, :]
    out["pex_t"] = np.ascontiguousarray(pex.T.reshape(NCK, 128, T).transpose(1, 0, 2))   # [128,4,T]

    mask = np.zeros((16, T, T), np.float32)
    for b in range(B):
        for h in range(H):
            mask[slot_of(b, h)] = self_mask[h]
    out["maskt"] = mask
    out["ident_bf"] = _bf(np.eye(128, dtype=np.float32))
    out["ident_f32"] = np.eye(128, dtype=np.float32)
    out["onesD_f32"] = np.full((128, 128), 1.0 / D, np.float32)  # folded 1/D

    out["content_code"] = np.ascontiguousarray(inp["content_code"])
    out["style_code"] = np.ascontiguousarray(inp["style_code"])
    out["init_state"] = np.ascontiguousarray(inp["init_state"])
    return out


def input_specs():
    bf, f32 = ml_dtypes.bfloat16, np.float32
    return {
        "w_qkv": ((128, L, NCK, 3 * D), bf), "w_out": ((128, L, NCK, D), bf),
        "w_ff1": ((128, L, NCK, DFF), bf), "w_ff2": ((128, L, NF, D), bf),
        "w_cav": ((128, L, NCK, D), bf), "w_cao": ((128, L, NCK, D), bf),
        "w_adain": ((128, NCK, 2 * D), bf),
        "w_mmr": ((128, NCK, M), bf), "w_row0": ((64, 2048), bf),
        "tab0_t": ((128, 12, T), f32), "pex_t": ((128, NCK, T), f32),
        "maskt": ((16, T, T), f32),
        "ident_bf": ((128, 128), bf), "ident_f32": ((128, 128), f32),
        "onesD_f32": ((128, 128), f32),
        "content_code": ((B, T, D), f32), "style_code": ((B, D), f32),
        "init_state": ((B, M), f32),
    }


# ---------------------------------------------------------------- builder
def build(tc, ins, outs, n_steps=T, dyn_loop=True, n_repeat=N_REPEAT):
    nc = tc.nc
    ctx = ExitStack()

    cp = ctx.enter_context(tc.tile_pool(name="consts", bufs=1))
    sp = ctx.enter_context(tc.tile_pool(name="state", bufs=1))
    ap_ = ctx.enter_context(tc.tile_pool(name="act", bufs=2))

    dma = nc.sync.dma_start
    TT = nc.vector.tensor_tensor
    TS = nc.vector.tensor_scalar
    TTR = nc.vector.tensor_tensor_reduce
    CP = nc.vector.tensor_copy
    ACP = nc.scalar.copy  # copy on ACT engine

    def load(pool, name):
        src = ins[name]
        t = pool.tile(list(src.shape), src.dtype, tag=name)
        dma(t[:], src[:])
        return t

    maskt = load(cp, "maskt")
    ident_bf = load(cp, "ident_bf"); ident_f32 = load(cp, "ident_f32")
    onesD = load(cp, "onesD_f32")
    tab0_t = load(cp, "tab0_t"); pex_t = load(cp, "pex_t")
    w_row0 = load(cp, "w_row0"); w_mmr = load(cp, "w_mmr")

    KT = sp.tile([128, L, NCK, B, T], BF16, tag="KT")
    V_row = sp.tile([128, L, D], BF16, tag="V_row")
    ca_addT = sp.tile([128, L, NCK, B, T], F32, tag="ca_addT")
    out_sb = sp.tile([64, B, T], F32, tag="out_sb")
    qblock = sp.tile([128, 8 * 16], BF16, tag="qblock")
    vcol = sp.tile([128, NCK, B, T], BF16, tag="vcol")
    rowb = sp.tile([64, B], BF16, tag="rowb")
    istb = sp.tile([64, B], BF16, tag="istb")

    # ================= preamble (once) =====================================
    with tc.tile_pool(name="pre", bufs=1) as prep, \
         tc.tile_pool(name="preps", bufs=3, space="PSUM") as preps:
        w_cav = load(prep, "w_cav"); w_cao = load(prep, "w_cao")
        w_adain = load(prep, "w_adain")

        cc = prep.tile([128, D], F32, tag="cc")
        dma(cc[:], ins["content_code"].rearrange("b t d -> (b t) d"))
        st = prep.tile([B, D], F32, tag="st")
        dma(st[:], ins["style_code"][:])
        ist = prep.tile([B, M], F32, tag="ist")
        dma(ist[:], ins["init_state"][:])

        ccT = prep.tile([128, NCK, B, T], F32, tag="ccT")
        for c in range(NCK):
            tp = preps.tile([128, 128], F32, tag="pps")
            nc.tensor.transpose(tp[:], cc[:, c * 128:(c + 1) * 128], ident_f32[:])
            CP(ccT[:, c, :, :], tp[:].rearrange("p (b t) -> p b t", b=B))

        mu = prep.tile([128, NCK, B], F32, tag="mu")
        nc.vector.tensor_reduce(mu[:], ccT[:], AX, ALU.add)
        sq = prep.tile([128, NCK, B, T], F32, tag="sqq")
        TT(sq[:], ccT[:], ccT[:], ALU.mult)
        s2 = prep.tile([128, NCK, B], F32, tag="s2")
        nc.vector.tensor_reduce(s2[:], sq[:], AX, ALU.add)
        nc.vector.tensor_scalar_mul(mu[:], mu[:], 1.0 / T)
        nc.vector.tensor_scalar_mul(s2[:], s2[:], 1.0 / T)
        mu2 = prep.tile([128, NCK, B], F32, tag="mu2")
        TT(mu2[:], mu[:], mu[:], ALU.mult)
        var = prep.tile([128, NCK, B], F32, tag="var")
        nc.vector.scalar_tensor_tensor(var[:], s2[:], EPS, mu2[:], ALU.add, ALU.subtract)
        rstd = prep.tile([128, NCK, B], F32, tag="rstd")
        nc.vector.reciprocal(rstd[:], var[:])
        nc.scalar.activation(rstd[:], rstd[:], ACTF.Sqrt)

        styT = prep.tile([128, NCK, B], F32, tag="styT")
        for c in range(NCK):
            tp = preps.tile([128, B], F32, tag="pps")
            nc.tensor.transpose(tp[:], st[:, c * 128:(c + 1) * 128], ident_f32[0:B, 0:B])
            CP(styT[:, c, :], tp[:])
        styb = prep.tile([128, NCK, B], BF16, tag="styb")
        CP(styb[:], styT[:])

        gd_ps = preps.tile([128, 8, B], F32, tag="pps")
        for mc in range(8):
            for kc in range(NCK):
                nc.tensor.matmul(gd_ps[:, mc, :], w_adain[:, kc, mc * 128:(mc + 1) * 128],
                                 styb[:, kc, :], start=(kc == 0), stop=(kc == NCK - 1))
        gd = prep.tile([128, 8, B], F32, tag="gdsb")
        CP(gd[:], gd_ps[:])

        memb = prep.tile([128, NCK, B, T], BF16, tag="memb")
        tmpm = prep.tile([128, NCK, B, T], F32, tag="tmpm")
        TT(tmpm[:], ccT[:], mu[:].broadcast_to((128, NCK, B, T)), ALU.subtract)
        TT(tmpm[:], tmpm[:], rstd[:].broadcast_to((128, NCK, B, T)), ALU.mult)
        TT(tmpm[:], tmpm[:], gd[:, 0:NCK, :].broadcast_to((128, NCK, B, T)), ALU.mult)
        TT(tmpm[:], tmpm[:], gd[:, NCK:8, :].broadcast_to((128, NCK, B, T)), ALU.add)
        CP(memb[:], tmpm[:])

        for l in range(L):
            cav_ps = preps.tile([128, NCK, B * T], F32, tag="pps")
            for mc in range(NCK):
                for kc in range(NCK):
                    nc.tensor.matmul(cav_ps[:, mc, :], w_cav[:, l, kc, mc * 128:(mc + 1) * 128],
                                     memb[:, kc, :, :].rearrange("p b t -> p (b t)"),
                                     start=(kc == 0), stop=(kc == NCK - 1))
            cavb = prep.tile([128, NCK, B, T], BF16, tag="cavb")
            CP(cavb[:], cav_ps[:].rearrange("p m (b t) -> p m b t", b=B))
            cao_ps = preps.tile([128, NCK, B * T], F32, tag="pps")
            for mc in range(NCK):
                for kc in range(NCK):
                    nc.tensor.matmul(cao_ps[:, mc, :], w_cao[:, l, kc, mc * 128:(mc + 1) * 128],
                                     cavb[:, kc, :, :].rearrange("p b t -> p (b t)"),
                                     start=(kc == 0), stop=(kc == NCK - 1))
            CP(ca_addT[:, l, :, :, :], cao_ps[:].rearrange("p m (b t) -> p m b t", b=B))

        ib_ps = preps.tile([64, B], F32, tag="pps")
        nc.tensor.transpose(ib_ps[:], ist[:], ident_f32[0:B, 0:B])
        CP(istb[:], ib_ps[:])

    # main weights / psum pools
    wp = ctx.enter_context(tc.tile_pool(name="weights", bufs=1))
    pp = ctx.enter_context(tc.tile_pool(name="ps", bufs=5, space="PSUM"))
    vp = ctx.enter_context(tc.tile_pool(name="vps", bufs=1, space="PSUM"))
    V_ps = []
    for l in range(L):
        V_ps.append(vp.tile([128, 512], F32, tag=f"vps{l}", name=f"vps{l}"))
    w_qkv = load(wp, "w_qkv"); w_out = load(wp, "w_out")
    w_ff1 = load(wp, "w_ff1"); w_ff2 = load(wp, "w_ff2")

    LNEXP = ACTF.Exp
    LNLOG = ACTF.Ln

    # ---------------- one decode step -------------------------------------
    def step(i):
        # ---- fused row matmul: qkv0 (mc 0..11) + emb (mc 12..15)
        q0_ps = pp.tile([128, 16, B], F32, tag="ps")
        for mc in range(16):
            nc.tensor.matmul(q0_ps[:, mc, :], w_row0[:, mc * 128:(mc + 1) * 128],
                             rowb[:], start=True, stop=True)

        x_res = ap_.tile([128, NCK, B], F32, tag="x0")
        TT(x_res[:], q0_ps[:, 12:16, :],
           pex_t[:, :, bass.ds(i, 1)].broadcast_to((128, NCK, B)), ALU.add)
        qkvb = ap_.tile([128, 12, B], BF16, tag="qkvb0")
        TT(qkvb[:], q0_ps[:, 0:12, :],
           tab0_t[:, :, bass.ds(i, 1)].broadcast_to((128, 12, B)), ALU.add)

        for l in range(L):
            # ---- q -> qblock; caches
            CP(qblock[0:64, 0::18].rearrange("p (c b) -> p c b", c=NCK),
               qkvb[0:64, 0:NCK, :])
            CP(qblock[64:128, 1::18].rearrange("p (c b) -> p c b", c=NCK),
               qkvb[64:128, 0:NCK, :])
            CP(KT[:, l, :, :, bass.ds(i, 1)].squeeze(), qkvb[:, 4:8, :])
            CP(vcol[:, :, :, bass.ds(i, 1)].squeeze(), qkvb[:, 8:12, :])

            # ---- scores (PE first), then V accumulation on PE
            sc_ps = pp.tile([16, T], F32, tag="ps")
            for c in range(NCK):
                for b in range(B):
                    e = 2 * c + b
                    nc.tensor.matmul(sc_ps[:], qblock[:, e * 16:(e + 1) * 16],
                                     KT[:, l, c, b, :], start=(e == 0), stop=(e == 7))
            for c in range(NCK):
                for b in range(B):
                    nc.tensor.matmul(V_ps[l][b * 64:(b + 1) * 64, c * 128:(c + 1) * 128],
                                     vcol[:, c, b, :], ident_bf[:],
                                     start=False, stop=True, skip_group_check=True)

            # ---- softmax; 1/S folded into transpose identity
            s_sb = ap_.tile([16, T], F32, tag="s_sb")
            TT(s_sb[:], sc_ps[:], maskt[:, bass.ds(i, 1), :].squeeze(), ALU.add)
            e_sb = ap_.tile([16, T], BF16, tag="e_sb")
            S = ap_.tile([16, 1], F32, tag="S")
            nc.scalar.activation(e_sb[:], s_sb[:], LNEXP, accum_out=S[:])
            # V_row refresh on ACT (parallel with DVE softmax chain)
            ACP(V_row[:, l, :], V_ps[l][:])
            Sinv = ap_.tile([16, 1], F32, tag="Sinv")
            nc.vector.reciprocal(Sinv[:], S[:])
            eyeS = ap_.tile([16, 16], BF16, tag="eyeS")
            TT(eyeS[:], ident_bf[0:16, 0:16],
               Sinv[:].broadcast_to((16, 16)), ALU.mult)

            pT_ps = pp.tile([128, 16], F32, tag="ps")
            nc.tensor.matmul(pT_ps[0:64, :], e_sb[:], eyeS[:], start=True, stop=True)
            nc.tensor.matmul(pT_ps[64:128, :], e_sb[:], eyeS[:], start=True, stop=True,
                             tile_position=(0, 64))
            pTs = ap_.tile([128, 16], BF16, tag="pTs")
            CP(pTs[:], pT_ps[:])

            # ---- o matmuls -> oT [128, (4c, 2b)]
            oT_ps = pp.tile([128, NCK, B], F32, tag="ps")
            for h in range(H):
                c, hp = h // 2, h % 2
                for b in range(B):
                    s = slot_of(b, h)
                    nc.tensor.matmul(
                        oT_ps[hp * 64:(hp + 1) * 64, c, b:b + 1],
                        V_row[b * 64:(b + 1) * 64, l, h * 64:(h + 1) * 64],
                        pTs[b * 64:(b + 1) * 64, s:s + 1],
                        start=True, stop=True, tile_position=(b * 64, hp * 64))
            oTs = ap_.tile([128, NCK, B], BF16, tag="oTs")
            CP(oTs[:], oT_ps[:])

            # ---- out projection
            pr_ps = pp.tile([128, NCK, B], F32, tag="ps")
            for mc in range(NCK):
                for kc in range(NCK):
                    nc.tensor.matmul(pr_ps[:, mc, :], w_out[:, l, kc, mc * 128:(mc + 1) * 128],
                                     oTs[:, kc, :], start=(kc == 0), stop=(kc == NCK - 1))

            # ---- LN1 (full, on-path): z1 = pr + x_res
            z1 = ap_.tile([128, NCK, B], F32, tag="z1")
            st1 = ap_.tile([128, 4], F32, tag="st1")
            TT(z1[:], pr_ps[:], x_res[:], ALU.add)
            nc.vector.tensor_reduce(st1[:, 0:2], z1[:].rearrange("p c b -> p b c"),
                                    AX, ALU.add)
            sq1 = ap_.tile([128, NCK, B], F32, tag="sq1")
            TT(sq1[:], z1[:], z1[:], ALU.mult)
            nc.vector.tensor_reduce(st1[:, 2:4], sq1[:].rearrange("p c b -> p b c"),
                                    AX, ALU.add)
            sm1_ps = pp.tile([128, 4], F32, tag="ps")
            nc.tensor.matmul(sm1_ps[:], onesD[:], st1[:], start=True, stop=True)
            sm1 = ap_.tile([128, 4], F32, tag="sm1")
            CP(sm1[:], sm1_ps[:])
            mu2t = ap_.tile([128, 2], F32, tag="mu2t")
            TT(mu2t[:], sm1[:, 0:2], sm1[:, 0:2], ALU.mult)
            var1 = ap_.tile([128, 2], F32, tag="var1")
            nc.vector.scalar_tensor_tensor(var1[:], sm1[:, 2:4], EPS,
                                           mu2t[:], ALU.add, ALU.subtract)
            lnv1 = ap_.tile([128, 2], F32, tag="lnv1")
            nc.scalar.activation(lnv1[:], var1[:], LNLOG)
            rstd1 = ap_.tile([128, 2], F32, tag="rstd1")
            nc.scalar.activation(rstd1[:], lnv1[:], LNEXP, scale=-0.5)
            x1 = ap_.tile([128, NCK, B], F32, tag="x1")
            for b in range(B):
                TS(x1[:, :, b], z1[:, :, b], sm1[:, b:b + 1], rstd1[:, b:b + 1],
                   ALU.subtract, ALU.mult)

            # ---- LN2 (deferred): z2 = x1 + ca
            z2 = ap_.tile([128, NCK, B], F32, tag="z2")
            st2 = ap_.tile([128, 4], F32, tag="st2")
            TT(z2[:], x1[:], ca_addT[:, l, :, :, bass.ds(i, 1)].squeeze(), ALU.add)
            nc.vector.tensor_reduce(st2[:, 0:2], z2[:].rearrange("p c b -> p b c"),
                                    AX, ALU.add)
            sq2 = ap_.tile([128, NCK, B], F32, tag="sq2")
            TT(sq2[:], z2[:], z2[:], ALU.mult)
            nc.vector.tensor_reduce(st2[:, 2:4], sq2[:].rearrange("p c b -> p b c"),
                                    AX, ALU.add)
            sm2_ps = pp.tile([128, 4], F32, tag="ps")
            nc.tensor.matmul(sm2_ps[:], onesD[:], st2[:], start=True, stop=True)
            sm2 = ap_.tile([128, 4], F32, tag="sm2")
            CP(sm2[:], sm2_ps[:])
            m2 = ap_.tile([128, NCK, B], BF16, tag="m2")
            for b in range(B):
                nc.vector.tensor_scalar_sub(m2[:, :, b], z2[:, :, b], sm2[:, b:b + 1])
            # rstd2 path (off critical: runs while ff1 streams)
            mu2t2 = ap_.tile([128, 2], F32, tag="mu2t2")
            TT(mu2t2[:], sm2[:, 0:2], sm2[:, 0:2], ALU.mult)
            var2 = ap_.tile([128, 2], F32, tag="var2")
            nc.vector.scalar_tensor_tensor(var2[:], sm2[:, 2:4], EPS,
                                           mu2t2[:], ALU.add, ALU.subtract)
            lnv2 = ap_.tile([128, 2], F32, tag="lnv2")
            nc.scalar.activation(lnv2[:], var2[:], LNLOG)
            rstd2 = ap_.tile([128, 2], F32, tag="rstd2")
            nc.scalar.activation(rstd2[:], lnv2[:], LNEXP, scale=-0.5)
            x2 = ap_.tile([128, NCK, B], F32, tag="x2")
            for b in range(B):
                TS(x2[:, :, b], z2[:, :, b], sm2[:, b:b + 1], rstd2[:, b:b + 1],
                   ALU.subtract, ALU.mult)

            # ---- FFN (rstd2 applied to ff1 output, fused with relu)
            ff_ps = pp.tile([128, NF, B], F32, tag="ps")
            for mc in range(NF):
                for kc in range(NCK):
                    nc.tensor.matmul(ff_ps[:, mc, :], w_ff1[:, l, kc, mc * 128:(mc + 1) * 128],
                                     m2[:, kc, :], start=(kc == 0), stop=(kc == NCK - 1))
            hb = ap_.tile([128, NF, B], BF16, tag="hb")
            for b in range(B):
                TS(hb[:, :, b], ff_ps[:, :, b], rstd2[:, b:b + 1], 0.0,
                   ALU.mult, ALU.max)

            f2_ps = pp.tile([128, NCK, B], F32, tag="ps")
            for mc in range(NCK):
                for kc in range(NF):
                    nc.tensor.matmul(f2_ps[:, mc, :], w_ff2[:, l, kc, mc * 128:(mc + 1) * 128],
                                     hb[:, kc, :], start=(kc == 0), stop=(kc == NF - 1))

            # ---- LN3 (deferred): z3 = f2 + x2
            z3 = ap_.tile([128, NCK, B], F32, tag="z3")
            st3 = ap_.tile([128, 4], F32, tag="st3")
            TT(z3[:], f2_ps[:], x2[:], ALU.add)
            nc.vector.tensor_reduce(st3[:, 0:2], z3[:].rearrange("p c b -> p b c"),
                                    AX, ALU.add)
            sq3 = ap_.tile([128, NCK, B], F32, tag="sq3")
            TT(sq3[:], z3[:], z3[:], ALU.mult)
            nc.vector.tensor_reduce(st3[:, 2:4], sq3[:].rearrange("p c b -> p b c"),
                                    AX, ALU.add)
            sm3_ps = pp.tile([128, 4], F32, tag="ps")
            nc.tensor.matmul(sm3_ps[:], onesD[:], st3[:], start=True, stop=True)
            sm3 = ap_.tile([128, 4], F32, tag="sm3")
            CP(sm3[:], sm3_ps[:])
            m3 = ap_.tile([128, NCK, B], BF16, tag="m3")
            for b in range(B):
                nc.vector.tensor_scalar_sub(m3[:, :, b], z3[:, :, b], sm3[:, b:b + 1])
            mu2t3 = ap_.tile([128, 2], F32, tag="mu2t3")
            TT(mu2t3[:], sm3[:, 0:2], sm3[:, 0:2], ALU.mult)
            var3 = ap_.tile([128, 2], F32, tag="var3")
            nc.vector.scalar_tensor_tensor(var3[:], sm3[:, 2:4], EPS,
                                           mu2t3[:], ALU.add, ALU.subtract)
            lnv3 = ap_.tile([128, 2], F32, tag="lnv3")
            nc.scalar.activation(lnv3[:], var3[:], LNLOG)
            rstd3 = ap_.tile([128, 2], F32, tag="rstd3")
            nc.scalar.activation(rstd3[:], lnv3[:], LNEXP, scale=-0.5)

            if l < L - 1:
                # next-layer QKV from m3, rstd3 applied after
                qkv_ps = pp.tile([128, 12, B], F32, tag="ps")
                for mc in range(12):
                    for kc in range(NCK):
                        nc.tensor.matmul(qkv_ps[:, mc, :],
                                         w_qkv[:, l + 1, kc, mc * 128:(mc + 1) * 128],
                                         m3[:, kc, :], start=(kc == 0), stop=(kc == NCK - 1))
                qkvb = ap_.tile([128, 12, B], BF16, tag="qkvb")
                for b in range(B):
                    nc.vector.tensor_scalar_mul(qkvb[:, :, b], qkv_ps[:, :, b],
                                                rstd3[:, b:b + 1])
                # x3 residual (off-path: needed after next attention)
                x3 = ap_.tile([128, NCK, B], F32, tag="x3")
                for b in range(B):
                    TS(x3[:, :, b], z3[:, :, b], sm3[:, b:b + 1], rstd3[:, b:b + 1],
                       ALU.subtract, ALU.mult)
                x_res = x3
            else:
                # ---- output row: r = (m3 @ mmr.T) * rstd3
                r_ps = pp.tile([64, B], F32, tag="ps")
                for kc in range(NCK):
                    nc.tensor.matmul(r_ps[:], w_mmr[:, kc, :], m3[:, kc, :],
                                     start=(kc == 0), stop=(kc == NCK - 1))
                TT(out_sb[:, :, bass.ds(i, 1)].squeeze(), r_ps[:], rstd3[0:64, :],
                   ALU.mult)
                TT(rowb[:], r_ps[:], rstd3[0:64, :], ALU.mult)

        # clear vcol column (holds layer-2's v; must be zero for later steps)
        nc.vector.memset(vcol[:, :, :, bass.ds(i, 1)].squeeze(), 0.0)

    # ---------------- repeat wrapper (timing) + decode loop ----------------
    def run_once():
        nc.vector.memset(KT[:], 0.0)
        nc.vector.memset(out_sb[:], 0.0)
        nc.vector.memset(qblock[:], 0.0)
        nc.vector.memset(vcol[:], 0.0)
        CP(rowb[:], istb[:])
        # zero-init V psum accumulators (vcol is all-zero here)
        for l in range(L):
            for c in range(NCK):
                for b in range(B):
                    nc.tensor.matmul(V_ps[l][b * 64:(b + 1) * 64, c * 128:(c + 1) * 128],
                                     vcol[:, c, b, :], ident_bf[:],
                                     start=True, stop=True, skip_group_check=True)
        if dyn_loop:
            with tc.For_i(0, n_steps, 1, hint_engines=(mybir.EngineType.PE,),
                          staggered_reset=True) as i:
                step(i)
        else:
            for i in range(n_steps):
                step(i)

    if n_repeat > 1:
        with tc.For_i(0, n_repeat, 1) as _r:
            run_once()
    else:
        run_once()

    # ---- final output
    fo_ps = pp.tile([128, 64], F32, tag="ps")
    nc.tensor.transpose(fo_ps[:], out_sb[:].rearrange("p b t -> p (b t)"),
                        ident_f32[0:64, 0:64])
    fo = ap_.tile([128, 64], F32, tag="fo")
    CP(fo[:], fo_ps[:])
    dma(outs["out"].rearrange("b t m -> (b t) m"), fo[:])

    ctx.close()


# ===================================================================== runner
_CACHE = {}


def _build_and_compile(n_repeat=N_REPEAT):
    key = f"nc{n_repeat}"
    if key in _CACHE:
        return _CACHE[key]
    import concourse.tile as _tile
    from concourse import bacc as _bacc
    nc = _bacc.Bacc("TRN2", target_bir_lowering=False, debug=False)
    ins, outs = {}, {}
    for name, (shape, dt) in input_specs().items():
        ins[name] = nc.dram_tensor(name, list(shape), mybir.dt.from_np(np.dtype(dt)),
                                   kind="ExternalInput").ap()
    outs["out"] = nc.dram_tensor("out", [B, T, M], mybir.dt.float32,
                                 kind="ExternalOutput").ap()
    with _tile.TileContext(nc) as tc:
        build(tc, ins, outs, n_steps=T, dyn_loop=True, n_repeat=n_repeat)
    nc.compile()
    _CACHE[key] = nc
    _CACHE["nc"] = nc if n_repeat == N_REPEAT else _CACHE.get("nc", nc)
    return nc


def kernel(**inputs):
    """Full (unsharded) inputs -> full output [B, T, M] float32."""
    from concourse.bass_utils import run_bass_kernel_spmd
    nc = _build_and_compile()
    dev_ins = prep_inputs(inputs)
    res = run_bass_kernel_spmd(nc, [dev_ins], core_ids=[0])
    return np.ascontiguousarray(res.results[0]["out"].astype(np.float32))
